# revision 31
# baseline (speedup 1.0000x reference)
"""CAPS attention Trainium2 kernel.

Self-contained: kernel(**inputs) -> np.ndarray, full (unsharded) in/out.
Shards 16 heads x 2 batches over 8 NeuronCores (2 heads, both batches per
core). W_q/k/p/gate/clock columns are tensor-parallel, W_c rows are
row-parallel; the host sums the 8 partial outputs.

Layout: projections produce [channel, t] (channels on partitions) so the
three cumsums run as native free-dim scans and softmax stats are
per-partition.

Precision (HW-measured): float32r matmul rounds operands to ~12 mantissa
bits (1.5e-4 rel err) but runs at bf16 speed (1 cycle/row at free-dim
>= 256), so all matmuls run f32r. Saturated softmax rows (t < 32, driven
by the branch-2 exp(cumsum) gate) have top-2 score gaps down to 9e-5, so
single-pass f32r would flip their argmax: the k projection runs 3 hi/lo
f32r passes everywhere, q/gate/clock do the same on chunk 0, and t-block 0
scores add q2/k2 residual passes. Everything else is single-pass; the
attention-value path (eS/eT2/v/outT/W_c) is bf16. Measured end-to-end
L2 2.4e-3 vs the 2e-2 gate.

Schedule: RoPE happens in the projection phase (DVE is idle there); row
max is skipped outside t-block 0 (|scaled scores| <= 0.26 there, far from
exp overflow; softmax is shift-invariant); softmax normalization is
applied post-attention via a transposed+broadcast 1/rowsum; stage_p2 of
the next (b,h) is emitted in small groups interleaved between stage_p3
t-blocks, and the output projection is folded in at tile-group
boundaries. k-side elementwise work runs on gpsimd.
"""

from contextlib import ExitStack

import numpy as np

import concourse.bass as bass  # noqa: F401  (bass types via bacc)
import concourse.tile as tile
from concourse import mybir, bacc
from concourse.bass_utils import run_bass_kernel_spmd

B, T, D, H = 2, 2048, 2048, 16
DH = 128          # head dim
HC = 2            # heads per core
C = HC * DH       # per-core channels = 256
NCORES = 8
TCH = 256         # t-chunk for projection phase
NCH = T // TCH
NTB = T // 128    # 16 t-sub-blocks
SCALE = float(1.0 / np.sqrt(np.float32(3 * DH)))

F32 = mybir.dt.float32
F32R = mybir.dt.float32r
BF16 = mybir.dt.bfloat16
AF = mybir.ActivationFunctionType
ALU = mybir.AluOpType
AX = mybir.AxisListType
SWAP32 = [(i + 1 if i % 2 == 0 else i - 1) for i in range(32)]


def _build_program():
    nc = bacc.Bacc(trn_type="TRN2")

    x_d = nc.dram_tensor("x", [B, T, D], F32, kind="ExternalInput")
    wq_d = nc.dram_tensor("wq", [D, C], F32, kind="ExternalInput")
    wk_d = nc.dram_tensor("wk", [D, C], F32, kind="ExternalInput")
    wg_d = nc.dram_tensor("wg", [D, C + HC], F32, kind="ExternalInput")
    wp_d = nc.dram_tensor("wp", [D, C], F32, kind="ExternalInput")
    wv_d = nc.dram_tensor("wv", [D, C], F32, kind="ExternalInput")
    wc_d = nc.dram_tensor("wc", [C, D], F32, kind="ExternalInput")
    cos_d = nc.dram_tensor("cosT", [DH, T], F32, kind="ExternalInput")
    sin_d = nc.dram_tensor("sinT", [DH, T], F32, kind="ExternalInput")
    id_d = nc.dram_tensor("ident", [128, 128], F32, kind="ExternalInput")
    out_d = nc.dram_tensor("out", [B, T, D], F32, kind="ExternalOutput")

    sp = {}
    for nm in ["q", "k", "g", "p"]:
        sp[nm] = nc.dram_tensor(f"sp_{nm}", [B, HC, DH, T], F32,
                                kind="Internal")
    sp_v = nc.dram_tensor("sp_v", [B, 128, NTB, C], BF16, kind="Internal")
    sp_clk = nc.dram_tensor("sp_clk", [B, HC * NTB, 128], F32, kind="Internal")

    with tile.TileContext(nc) as tc:
        with ExitStack() as ctx:
            perm = ctx.enter_context(tc.tile_pool(name="perm", bufs=1))

            id_t = perm.tile([128, 128], F32, tag="id_t")
            nc.sync.dma_start(id_t[:], id_d[:])
            id_b = perm.tile([128, 128], BF16, tag="id_b")
            nc.vector.tensor_copy(id_b[:], id_t[:])
            one_b = nc.const_aps.scalar_like(1.0, id_t[:])

            clkcol = [perm.tile([128, HC, NTB], F32, tag=f"clkcol{b}",
                                name=f"clkcol{b}") for b in range(B)]

            # ================= P1: projections (+RoPE) =================
            # All matmuls are f32r. The k projection runs 3 hi/lo passes
            # (w_hi*x_hi + w_lo*x_hi + w_hi*x_lo) everywhere, and q/gate/
            # clock do the same on chunk 0 only: saturated softmax rows
            # (t<32) have top-2 score gaps down to ~9e-5, so single-pass
            # f32r (~2e-4) would flip their argmax.
            with ExitStack() as p1:
                wp_ = p1.enter_context(tc.tile_pool(name="wpool", bufs=1))
                tp_ = p1.enter_context(tc.tile_pool(name="p1t", bufs=2))
                tp1 = p1.enter_context(tc.tile_pool(name="p1s", bufs=1))
                psum = p1.enter_context(
                    tc.tile_pool(name="psum1", bufs=1, space="PSUM"))
                pstr_p = p1.enter_context(
                    tc.tile_pool(name="psum1t", bufs=2, space="PSUM"))

                cos_t = wp_.tile([DH, T], F32, tag="cos_t")
                sin_t = wp_.tile([DH, T], F32, tag="sin_t")
                nc.sync.dma_start(cos_t[:], cos_d[:])
                nc.sync.dma_start(sin_t[:], sin_d[:])

                def load_w(wd, cols, pfx):
                    w = wp_.tile([128, 16, C + HC], F32R, tag=f"w_{pfx}",
                                 name=f"w_{pfx}")
                    wr = wd[:, :].rearrange("(o i) c -> i o c", i=128)
                    for hf in range(4):
                        stg = wp_.tile([128, 4, C + HC], F32, tag="w_stage",
                                       bufs=2)
                        s = stg[:, :, :cols]
                        nc.sync.dma_start(s, wr[:, hf * 4:(hf + 1) * 4, :])
                        nc.vector.tensor_copy(
                            w[:, hf * 4:(hf + 1) * 4, :cols], s)
                    return w

                def load_w_lo(wd, cols, whi, pfx, tag="w_lo"):
                    # lo = round_f32r(w - round_f32r(w)), re-staged from HBM
                    wl = wp_.tile([128, 16, C + HC], F32R, tag=tag,
                                  name=f"wlo_{pfx}")
                    wr = wd[:, :].rearrange("(o i) c -> i o c", i=128)
                    for hf in range(4):
                        stg = wp_.tile([128, 4, C + HC], F32, tag="w_stage",
                                       bufs=2)
                        s = stg[:, :, :cols]
                        nc.sync.dma_start(s, wr[:, hf * 4:(hf + 1) * 4, :])
                        nc.vector.tensor_sub(
                            wl[:, hf * 4:(hf + 1) * 4, :cols], s,
                            whi[:, hf * 4:(hf + 1) * 4, :cols].bitcast(F32))
                    return wl

                w_q = load_w(wq_d, C, "q")
                w_k = load_w(wk_d, C, "k")
                w_g = load_w(wg_d, C + HC, "g")
                w_p = load_w(wp_d, C, "p")
                w_v = load_w(wv_d, C, "v")
                w_k_lo = load_w_lo(wk_d, C, w_k, "k", tag="w_klo")

                for b in range(B):
                    for chk in range(NCH):
                        xT = tp_.tile([128, 16, TCH], F32R, tag="xT")
                        xL = tp1.tile([128, 16, TCH], F32R, tag="xL")
                        for ts in range(TCH // 128):
                            xt = tp1.tile([128, D], F32, tag="x_in")
                            r0 = chk * TCH + ts * 128
                            nc.sync.dma_start(xt[:], x_d[b, r0:r0 + 128, :])
                            for kg in range(4):
                                pst = pstr_p.tile([128, 512], F32,
                                                  tag="ps_tr")
                                for j in range(4):
                                    ko = kg * 4 + j
                                    nc.tensor.transpose(
                                        pst[:, j * 128:(j + 1) * 128],
                                        xt[:, ko * 128:(ko + 1) * 128],
                                        id_t[:])
                                dsl = (slice(None), slice(kg * 4, kg * 4 + 4),
                                       slice(ts * 128, (ts + 1) * 128))
                                srcv = pst[:].rearrange("p (a b) -> p a b",
                                                        a=4)
                                nc.scalar.copy(xT[dsl], srcv)
                                nc.vector.tensor_sub(xL[dsl], srcv,
                                                     xT[dsl].bitcast(F32))

                        tsl = slice(chk * TCH, (chk + 1) * TCH)
                        hp = (chk == 0)  # high-precision chunk
                        for nm, w in [("q", w_q), ("k", w_k), ("g", w_g),
                                      ("p", w_p)]:
                            full = (nm == "k") or (hp and nm in ("q", "g"))
                            for h in range(HC):
                                hsl = slice(h * DH, (h + 1) * DH)
                                ps = psum.tile([128, TCH], F32, tag="ps_proj")
                                for ko in range(16):
                                    nc.tensor.matmul(
                                        ps[:], w[:, ko, hsl], xT[:, ko, :],
                                        start=(ko == 0),
                                        stop=(ko == 15 and not full))
                                if full:
                                    wl = (w_k_lo if nm == "k"
                                          else load_w_lo(
                                              wq_d if nm == "q" else wg_d,
                                              C + (HC if nm == "g" else 0),
                                              w, nm + f"{b}"))
                                    for ko in range(16):
                                        nc.tensor.matmul(
                                            ps[:], wl[:, ko, hsl],
                                            xT[:, ko, :],
                                            start=False, stop=False,
                                            skip_group_check=True)
                                    for ko in range(16):
                                        nc.tensor.matmul(
                                            ps[:], w[:, ko, hsl],
                                            xL[:, ko, :],
                                            start=False, stop=(ko == 15),
                                            skip_group_check=True)
                                    if nm == "g" and h == HC - 1:
                                        loc_wg_lo = wl
                                prc = tp_.tile([128, TCH], F32, tag="prcp")
                                nc.scalar.copy(prc[:], ps[:])
                                if nm in ("q", "k"):
                                    # RoPE (t-pointwise; sin_t carries sign)
                                    sh = tp1.tile([128, TCH], F32, tag="shp")
                                    nc.vector.stream_shuffle(sh[:], prc[:],
                                                             SWAP32)
                                    nc.vector.tensor_mul(sh[:], sh[:],
                                                         sin_t[:, tsl])
                                    nc.vector.tensor_mul(prc[:], prc[:],
                                                         cos_t[:, tsl])
                                    nc.vector.tensor_add(prc[:], prc[:],
                                                         sh[:])
                                nc.sync.dma_start(sp[nm][b, h, :, tsl], prc[:])
                        for ts in range(TCH // 128):
                            psc = psum.tile([128, HC], F32, tag="ps_clk")
                            xsl = slice(ts * 128, (ts + 1) * 128)
                            for ko in range(16):
                                nc.tensor.matmul(
                                    psc[:], xT[:, ko, xsl],
                                    w_g[:, ko, C:C + HC],
                                    start=(ko == 0), stop=(ko == 15 and
                                                           not hp))
                            if hp:
                                for ko in range(16):
                                    nc.tensor.matmul(
                                        psc[:], xT[:, ko, xsl],
                                        loc_wg_lo[:, ko, C:C + HC],
                                        start=False, stop=False,
                                        skip_group_check=True)
                                for ko in range(16):
                                    nc.tensor.matmul(
                                        psc[:], xL[:, ko, xsl],
                                        w_g[:, ko, C:C + HC],
                                        start=False, stop=(ko == 15),
                                        skip_group_check=True)
                            to = chk * (TCH // 128) + ts
                            nc.vector.tensor_copy(clkcol[b][:, :, to], psc[:])
                        for ts in range(TCH // 128):
                            ps = psum.tile([128, C], F32, tag="ps_v")
                            for ko in range(16):
                                nc.tensor.matmul(
                                    ps[:], xT[:, ko, ts * 128:(ts + 1) * 128],
                                    w_v[:, ko, :C],
                                    start=(ko == 0), stop=(ko == 15))
                            vcp = tp1.tile([128, C], BF16, tag="vcp")
                            nc.vector.tensor_copy(vcp[:], ps[:])
                            to = chk * (TCH // 128) + ts
                            nc.sync.dma_start(sp_v[b, :, to, :], vcp[:])

                    if chk == NCH - 1:
                        ctr = psum.tile([128, 128], F32, tag="ps_ctr")
                        nc.tensor.transpose(
                            ctr[:HC * NTB, :],
                            clkcol[b][:].rearrange("p h o -> p (h o)"),
                            id_t[:])
                        ctb = tp1.tile([HC * NTB, 128], F32, tag="ctb")
                        nc.vector.tensor_copy(ctb[:], ctr[:HC * NTB, :])
                        nc.sync.dma_start(sp_clk[b], ctb[:])


            # ================= work phase =================
            with ExitStack() as wk:
                wkp = wk.enter_context(tc.tile_pool(name="work", bufs=1))
                dbp = wk.enter_context(tc.tile_pool(name="dbuf", bufs=2))
                sgl = wk.enter_context(tc.tile_pool(name="sgl", bufs=1))
                psum = wk.enter_context(
                    tc.tile_pool(name="psum2", bufs=1, space="PSUM"))

                wc_r = wkp.tile([128, HC, D], BF16, tag="wc_r")
                for h in range(HC):
                    wch = dbp.tile([128, 1024], F32, tag="fin", name="wch")
                    nc.sync.dma_start(wch[:], wc_d[h * 128:(h + 1) * 128,
                                                   0:1024])
                    nc.vector.tensor_copy(wc_r[:, h, 0:1024], wch[:])
                    wch2 = dbp.tile([128, 1024], F32, tag="fin", name="wch2")
                    nc.sync.dma_start(wch2[:], wc_d[h * 128:(h + 1) * 128,
                                                    1024:2048])
                    nc.vector.tensor_copy(wc_r[:, h, 1024:2048], wch2[:])

                v_r = {}
                outT = {}

                def stage_p2_groups(b, h, st):
                    """Closures emitted spread across p3 of the previous
                    pair. Scratch buffers are lifetime-planned:
                    sA=qR (becomes q2 in place), sB=kR (becomes k2),
                    sC=r_clk, sD=r_ccs->r_lclk->pcs->gT, sE=pT->gcs."""
                    loc = {}

                    def g_start():
                        if h == 0:
                            outT[b] = wkp.tile([128, HC, T], BF16, tag="outT",
                                               name=f"outT{b}")
                        v_r[(b, h)] = dbp.tile([128, NTB, DH], BF16,
                                               tag="v_r", name="v_r")
                        nc.sync.dma_start(
                            v_r[(b, h)][:],
                            sp_v[b][:, :, h * DH:(h + 1) * DH])
                        loc["qR"] = sgl.tile([128, T], F32, tag="sA",
                                             name="qR")
                        nc.sync.dma_start(loc["qR"][:], sp["q"][b, h])
                        loc["kR"] = sgl.tile([128, T], F32, tag="sB",
                                             name="kR")
                        nc.sync.dma_start(loc["kR"][:], sp["k"][b, h])

                    def g_clock():
                        brow = sgl.tile([1, T], F32, tag="brw", name="brow")
                        nc.sync.dma_start(
                            brow[:],
                            sp_clk[b].rearrange("(h o) i -> h (o i)",
                                                h=HC)[h:h + 1, :])
                        nc.scalar.activation(brow[:], brow[:], AF.Exp)
                        nc.scalar.activation(brow[:], brow[:], AF.Ln,
                                             bias=one_b[:1])
                        nc.vector.tensor_scalar_add(brow[:], brow[:], 1e-6)
                        r_clk = sgl.tile([128, T], F32, tag="sC",
                                         name="r_clk")
                        nc.gpsimd.partition_broadcast(r_clk[:], brow[:])
                        loc["brow"] = brow
                        loc["r_clk"] = r_clk

                    def g_ccs():
                        r_ccs = sgl.tile([128, T], F32, tag="sD",
                                         name="r_ccs")
                        nc.vector.tensor_tensor_scan(
                            r_ccs[:], loc["r_clk"][:], loc["r_clk"][:],
                            0.0, ALU.add, ALU.bypass)
                        nc.vector.reciprocal_approx_fast(r_ccs[:], r_ccs[:])
                        loc["r_ccs"] = r_ccs

                    def g_br3():
                        st["q3"] = sgl.tile([128, T], F32R, tag="br_q3",
                                            name="q3")
                        st["k3"] = sgl.tile([128, T], F32R, tag="br_k3",
                                            name="k3")
                        nc.vector.scalar_tensor_tensor(
                            st["q3"][:], loc["qR"][:], SCALE,
                            loc["r_ccs"][:], ALU.mult, ALU.mult)
                        nc.gpsimd.tensor_mul(st["k3"][:], loc["kR"][:],
                                             loc["r_clk"][:])

                    def g_pT():
                        nc.scalar.activation(loc["brow"][:], loc["brow"][:],
                                             AF.Ln)
                        r_lclk = sgl.tile([128, T], F32, tag="sD",
                                          name="r_lclk")
                        nc.gpsimd.partition_broadcast(r_lclk[:],
                                                      loc["brow"][:])
                        pT = sgl.tile([128, T], F32, tag="sE", name="pT")
                        nc.sync.dma_start(pT[:], sp["p"][b, h])
                        nc.gpsimd.tensor_add(pT[:], pT[:], r_lclk[:])
                        loc["pT"] = pT

                    def g_pexp():
                        nmx1 = dbp.tile([128, 1], F32, tag="pmax")
                        nc.vector.tensor_reduce(nmx1[:], loc["pT"][:],
                                                axis=AX.X,
                                                op=ALU.max, negate=True)
                        nc.scalar.activation(loc["pT"][:], loc["pT"][:],
                                             AF.Exp, bias=nmx1[:, 0:1])

                    def g_pcs():
                        pcs = sgl.tile([128, T], F32, tag="sD", name="pcs")
                        nc.vector.tensor_tensor_scan(
                            pcs[:], loc["pT"][:], loc["pT"][:],
                            0.0, ALU.add, ALU.bypass)
                        nc.vector.tensor_scalar_add(pcs[:], pcs[:], 1e-8)
                        nc.vector.reciprocal_approx_fast(pcs[:], pcs[:])
                        loc["pcs"] = pcs

                    def g_br1():
                        st["q1"] = dbp.tile([128, T], F32R, tag="br_q1",
                                            name="q1")
                        st["k1"] = dbp.tile([128, T], F32R, tag="br_k1",
                                            name="k1")
                        nc.vector.scalar_tensor_tensor(
                            st["q1"][:], loc["qR"][:], SCALE,
                            loc["pcs"][:], ALU.mult, ALU.mult)
                        nc.gpsimd.tensor_mul(st["k1"][:], loc["kR"][:],
                                             loc["pT"][:])

                    def g_gT():
                        gT = sgl.tile([128, T], F32, tag="sD", name="gT")
                        nc.sync.dma_start(gT[:], sp["g"][b, h])
                        nc.scalar.activation(gT[:], gT[:], AF.Exp)
                        nc.scalar.activation(gT[:], gT[:], AF.Ln,
                                             bias=one_b[:128])
                        nc.gpsimd.tensor_mul(gT[:], gT[:], loc["r_clk"][:])
                        loc["gT"] = gT

                    def g_gcs():
                        # y = cumsum(softplus(g)*clock) >= 0;
                        # gj_cp = exp(clip(-y,-50,40)) = exp(-clip(y,-40,50))
                        gcs = sgl.tile([128, T], F32, tag="sE", name="gcs")
                        nc.vector.tensor_tensor_scan(
                            gcs[:], loc["gT"][:], loc["gT"][:],
                            0.0, ALU.add, ALU.bypass)
                        nc.vector.tensor_scalar(gcs[:], gcs[:], 50.0, -40.0,
                                                ALU.min, ALU.max)
                        nc.scalar.activation(gcs[:], gcs[:], AF.Exp,
                                             scale=-1.0)
                        loc["gcs"] = gcs

                    def g_br2q():
                        st["q2"] = dbp.tile([128, T], F32R, tag="br_q2",
                                            name="q2")
                        nc.vector.scalar_tensor_tensor(
                            st["q2"][:], loc["qR"][:], SCALE,
                            loc["gcs"][:], ALU.mult, ALU.mult)
                        # f32r residual of q2 for the saturated t-block
                        q2f = dbp.tile([128, 128], F32, tag="q2f")
                        nc.vector.scalar_tensor_tensor(
                            q2f[:], loc["qR"][:, 0:128], SCALE,
                            loc["gcs"][:, 0:128], ALU.mult, ALU.mult)
                        st["q2lo"] = dbp.tile([128, 128], F32R, tag="q2lo",
                                              name="q2lo")
                        nc.vector.tensor_sub(st["q2lo"][:], q2f[:],
                                             st["q2"][:, 0:128].bitcast(F32))

                    def g_br2k():
                        st["k2"] = dbp.tile([128, T], F32R, tag="br_k2",
                                            name="k2")
                        nc.vector.tensor_scalar_add(loc["gcs"][:],
                                                    loc["gcs"][:], 1e-8)
                        nc.vector.reciprocal_approx_fast(loc["gcs"][:],
                                                         loc["gcs"][:])
                        k2f = sgl.tile([128, T], F32, tag="sD", name="k2f")
                        nc.gpsimd.tensor_mul(k2f[:], loc["kR"][:],
                                             loc["gcs"][:])
                        nc.vector.tensor_copy(st["k2"][:], k2f[:])
                        st["k2lo"] = sgl.tile([128, T], F32R, tag="k2lo",
                                              name="k2lo")
                        nc.vector.tensor_sub(st["k2lo"][:], k2f[:],
                                             st["k2"][:].bitcast(F32))

                    return [g_start, g_clock, g_pT, g_pexp, g_pcs,
                            g_br1, g_gT, g_gcs, g_br2q, g_br2k, g_ccs,
                            g_br3]

                def stage_p3(b, h, st, extras):
                    def ap(nm, sl):
                        return st[nm][:, sl]

                    rs_all = wkp.tile([128, NTB], F32, tag="rs_all")
                    eT2 = {}
                    eSb = {}
                    ex = list(extras)

                    def mm_softmax(tb):
                        tsl = slice(tb * 128, (tb + 1) * 128)
                        pA = psum.tile([128, 1024], F32, tag="ps_A")
                        pB = psum.tile([128, 1024], F32, tag="ps_B")
                        for i, p in [(0, pA), (1, pA), (2, pB), (3, pB)]:
                            osl = slice((i % 2) * 512, (i % 2) * 512 + 512)
                            ssl = slice(i * 512, (i + 1) * 512)
                            nc.tensor.matmul(p[:, osl], ap("q1", tsl),
                                             ap("k1", ssl),
                                             start=True, stop=False)
                            nc.tensor.matmul(p[:, osl], ap("q2", tsl),
                                             ap("k2", ssl),
                                             start=False, stop=False,
                                             skip_group_check=True)
                            if tb == 0:
                                nc.tensor.matmul(p[:, osl], ap("q2", tsl),
                                                 st["k2lo"][:, ssl],
                                                 start=False, stop=False,
                                                 skip_group_check=True)
                                nc.tensor.matmul(p[:, osl], st["q2lo"][:],
                                                 ap("k2", ssl),
                                                 start=False, stop=False,
                                                 skip_group_check=True)
                            nc.tensor.matmul(p[:, osl], ap("q3", tsl),
                                             ap("k3", ssl),
                                             start=False, stop=True,
                                             skip_group_check=True)
                        e = dbp.tile([128, T], BF16, tag="eSb", name="eSb")
                        eSb[tb] = e
                        sm2 = dbp.tile([128, 2], F32, tag="sm2")
                        if tb == 0:
                            # saturated rows (|score| up to ~2e8) only exist
                            # at t<30; exact row-max here. Elsewhere scores
                            # are bounded by ~0.26, so exp needs no bias.
                            mx2 = dbp.tile([128, 2], F32, tag="mx2")
                            nc.vector.tensor_reduce(mx2[:, 0:1], pA[:],
                                                    axis=AX.X, op=ALU.max)
                            nc.vector.tensor_reduce(mx2[:, 1:2], pB[:],
                                                    axis=AX.X, op=ALU.max)
                            nmx = dbp.tile([128, 1], F32, tag="nmx")
                            nc.vector.tensor_reduce(nmx[:], mx2[:], axis=AX.X,
                                                    op=ALU.max, negate=True)
                            nc.scalar.activation(e[:, 0:1024], pA[:], AF.Exp,
                                                 bias=nmx[:, 0:1],
                                                 accum_out=sm2[:, 0:1])
                            nc.scalar.activation(e[:, 1024:2048], pB[:],
                                                 AF.Exp, bias=nmx[:, 0:1],
                                                 accum_out=sm2[:, 1:2])
                        else:
                            nc.scalar.activation(e[:, 0:1024], pA[:], AF.Exp,
                                                 accum_out=sm2[:, 0:1])
                            nc.scalar.activation(e[:, 1024:2048], pB[:],
                                                 AF.Exp,
                                                 accum_out=sm2[:, 1:2])
                        rs = dbp.tile([128, 1], F32, tag="rs")
                        nc.vector.tensor_reduce(rs[:], sm2[:],
                                                axis=AX.X, op=ALU.add)
                        nc.vector.reciprocal_approx_fast(
                            rs_all[:, tb:tb + 1], rs[:])

                    def trcopy(tb):
                        tg, tj = tb // 2, tb % 2
                        if tj == 0:
                            eT2[tg] = wkp.tile([128, NTB, 256], BF16,
                                               tag="eT2", name="eT2")
                        e = eSb.pop(tb)
                        for sg in range(4):
                            pstr = psum.tile([128, 512], BF16, tag="ps_eT")
                            for j in range(4):
                                so = sg * 4 + j
                                nc.tensor.transpose(
                                    pstr[:, j * 128:(j + 1) * 128],
                                    e[:, so * 128:(so + 1) * 128],
                                    id_b[:])
                            dst = eT2[tg][:, sg * 4:(sg + 1) * 4,
                                          tj * 128:(tj + 1) * 128]
                            src = pstr[:].rearrange("p (a b) -> p a b", a=4)
                            nc.vector.tensor_copy(dst, src)

                    def attn_v(tg):
                        # 1/rowsum per t-block, transposed one column at a
                        # time so each lands on partition 0, then broadcast
                        ps_rr = psum.tile([1, 256], F32, tag="ps_rr")
                        for j in range(2):
                            col = tg * 2 + j
                            nc.tensor.transpose(
                                ps_rr[:, j * 128:(j + 1) * 128],
                                rs_all[:, col:col + 1], id_t[:])
                        rr4 = dbp.tile([1, 256], F32, tag="rr4")
                        nc.vector.tensor_copy(rr4[:], ps_rr[:])
                        rrb = dbp.tile([128, 256], F32, tag="rrb")
                        nc.gpsimd.partition_broadcast(rrb[:], rr4[:])

                        pav = psum.tile([128, 256], F32, tag="ps_av")
                        e = eT2.pop(tg)
                        for so in range(NTB):
                            nc.tensor.matmul(pav[:], v_r[(b, h)][:, so, :],
                                             e[:, so, :],
                                             start=(so == 0), stop=(so == 15))
                        nc.vector.tensor_mul(
                            outT[b][:, h, tg * 256:(tg + 1) * 256],
                            pav[:], rrb[:])

                    def p4_unit(tb):
                        tsl = slice(tb * 128, (tb + 1) * 128)
                        for half in range(2):
                            fin = dbp.tile([128, 1024], F32, tag="fin",
                                           name="fin")
                            for nck in range(2):
                                n0 = half * 1024 + nck * 512
                                nsl = slice(n0, n0 + 512)
                                psf = psum.tile([128, 512], F32, tag="ps_fin")
                                for hh in range(HC):
                                    nc.tensor.matmul(
                                        psf[:], outT[b][:, hh, tsl],
                                        wc_r[:, hh, nsl],
                                        start=(hh == 0), stop=(hh == 1))
                                dst = fin[:, nck * 512:(nck + 1) * 512]
                                if nck % 2 == 0:
                                    nc.scalar.copy(dst, psf[:])
                                else:
                                    nc.vector.tensor_copy(dst, psf[:])
                            nc.sync.dma_start(
                                out_d[b, tsl, half * 1024:half * 1024 + 1024],
                                fin[:])

                    for tb in range(NTB):
                        mm_softmax(tb)
                        if tb > 0:
                            trcopy(tb - 1)
                        if tb % 2 == 0 and tb > 0:
                            attn_v(tb // 2 - 1)
                            if h == HC - 1 and tb >= 4 and tb % 2 == 0:
                                for t2 in range(tb - 4, tb - 2):
                                    p4_unit(t2)
                        if ex:
                            ex.pop(0)()
                    trcopy(NTB - 1)
                    attn_v(NTB // 2 - 1)
                    if h == HC - 1:
                        for t2 in range(NTB - 4, NTB):
                            p4_unit(t2)
                    for g in ex:
                        g()

                pairs = [(b, h) for b in range(B) for h in range(HC)]
                prev = None
                for i, (b, h) in enumerate(pairs):
                    st = {}
                    groups = stage_p2_groups(b, h, st)
                    if prev is None:
                        # dependency order (g_ccs/g_br3 pulled early; the
                        # tail ordering only matters for cross-pair reuse)
                        g = groups
                        first = [g[0], g[1], g[10], g[11], g[2], g[3],
                                 g[4], g[5], g[6], g[7], g[8], g[9]]
                        for gg in first:
                            gg()
                    else:
                        pb, ph, pst = prev
                        stage_p3(pb, ph, pst, groups)
                    prev = (b, h, st)
                pb, ph, pst = prev
                stage_p3(pb, ph, pst, [])

    nc.compile()
    return nc


_PROGRAM_CACHE = None


def _get_program():
    global _PROGRAM_CACHE
    if _PROGRAM_CACHE is None:
        _PROGRAM_CACHE = _build_program()
    return _PROGRAM_CACHE


def _host_tables():
    d = DH
    inv_freq = 1.0 / (np.float32(10000.0) **
                      (np.arange(0, d, 2, dtype=np.float32) / np.float32(d)))
    t = np.arange(T, dtype=np.float32)
    freqs = t[:, None] * inv_freq[None, :].astype(np.float32)
    emb = np.concatenate([freqs, freqs], axis=-1).astype(np.float32)
    cosT = np.ascontiguousarray(np.cos(emb).astype(np.float32).T)
    sgn = np.where(np.arange(d) % 2 == 0, -1.0, 1.0).astype(np.float32)[:, None]
    sinT = np.ascontiguousarray(np.sin(emb).astype(np.float32).T * sgn)
    return cosT, sinT


def kernel(x, W_q, W_k, W_v, W_gate, W_p, W_clock, W_c, _trace=False,
           _core_ids=None):
    x = np.ascontiguousarray(np.asarray(x, dtype=np.float32))
    cosT, sinT = _host_tables()
    ident = np.eye(128, dtype=np.float32)
    core_ids = list(range(NCORES)) if _core_ids is None else list(_core_ids)

    in_maps = []
    for c in core_ids:
        c0 = c * C
        wg_ext = np.concatenate(
            [np.asarray(W_gate)[:, c0:c0 + C],
             np.asarray(W_clock)[:, c * HC:(c + 1) * HC]], axis=1)
        in_maps.append({
            "x": x,
            "wq": np.ascontiguousarray(np.asarray(W_q)[:, c0:c0 + C]),
            "wk": np.ascontiguousarray(np.asarray(W_k)[:, c0:c0 + C]),
            "wg": np.ascontiguousarray(wg_ext.astype(np.float32)),
            "wp": np.ascontiguousarray(np.asarray(W_p)[:, c0:c0 + C]),
            "wv": np.ascontiguousarray(np.asarray(W_v)[:, c0:c0 + C]),
            "wc": np.ascontiguousarray(np.asarray(W_c)[c0:c0 + C, :]),
            "cosT": cosT, "sinT": sinT, "ident": ident,
        })

    nc = _get_program()
    res = run_bass_kernel_spmd(nc, in_maps, core_ids=core_ids, trace=_trace)
    out = np.zeros((B, T, D), dtype=np.float64)
    for r in res.results:
        out += r["out"].astype(np.float64)
    kernel._last_result = res
    return out.astype(np.float32)
